# revision 21
# baseline (speedup 1.0000x reference)
"""Causal MHA on 8 TRN2 NeuronCores.

Sharding: 8 cores = 4 batches x 2 head-groups (6 heads each).  Every core
runs an identical graph: QKV projections for its batch + head-group, causal
attention (max-free softmax, scores kept key-major so exp output feeds the
PV matmul directly with no transpose), and a partial output projection over
its 6 heads.  Host sums the two partials per batch (+ b_O).  No collectives.

Schedule: V projection up front, then Q/K projections for head-pair g+1 are
interleaved into the attention of pair g, keeping TensorE dense while
ScalarE chews on exp (which keeps the HAM clock-gate at 2.4 GHz).  A 2-deep
software pipeline (PV trails QK) hides exp latency inside each unit, and
each unit's softmax epilogue is emitted during the next unit's QK prologue.

All matmul operands are bf16 (1 cycle/row at any N, FWL weight loads);
PSUM accumulation stays fp32.  exp is max-free (scores are bounded), and
an extra all-ones V' column makes the PV matmul emit softmax denominators.
"""

import sys

for _p in ("/opt/trn_rl_repo",):
    if _p not in sys.path:
        sys.path.insert(0, _p)

import ml_dtypes
import numpy as np

import concourse.bass as bass
import concourse.mybir as mybir
import concourse.tile as tile
from concourse import bacc
from concourse.bass_utils import run_bass_kernel_spmd

F32 = mybir.dt.float32
BF16 = mybir.dt.bfloat16
BF = ml_dtypes.bfloat16

P = 128
EMB = 768
NKT = EMB // P          # 6 emb tiles
S = 2048
NTT = S // P            # 16 token tiles
DH = 64
NH = 6                  # heads per core
NG = NH // 2            # dh-groups of 128 (2 heads stacked)
TC = 512
NTC = S // TC           # 4 token chunks
VW = NH * (DH + 1)      # 390: V' row width per token tile
N_CORES = 8
PIPE = 2                # PV trails QK by this many key tiles

_graph_cache = {}

_TRI = np.triu(np.ones((P, P))).astype(BF)  # tri[k, q] = 1 iff k <= q
_ONES = np.ones((1, P), dtype=BF)


def _widen_wv(wv):  # [NH, EMB, DH] -> [EMB, NH*(DH+1)], zero 65th cols
    out = np.zeros((EMB, VW), dtype=BF)
    out.reshape(EMB, NH, DH + 1)[:, :, :DH] = wv.transpose(1, 0, 2).astype(BF)
    return out


def _widen_bv(bv):  # [NH, DH] -> [1, NH*(DH+1)], 1.0 at 65th cols
    out = np.zeros((1, VW), dtype=BF)
    out.reshape(NH, DH + 1)[:, :DH] = bv.astype(BF)
    out.reshape(NH, DH + 1)[:, DH] = 1.0
    return out


def build_graph():
    nc = bacc.Bacc(None, target_bir_lowering=False)

    xT_d = nc.dram_tensor("xT", [EMB, S], BF16, kind="ExternalInput")
    WQ_d = nc.dram_tensor("WQ", [EMB, NH * DH], BF16, kind="ExternalInput")
    WK_d = nc.dram_tensor("WK", [EMB, NH * DH], BF16, kind="ExternalInput")
    WV_d = nc.dram_tensor("WV", [EMB, VW], BF16, kind="ExternalInput")
    WO_d = nc.dram_tensor("WO", [NH * DH, EMB], BF16, kind="ExternalInput")
    bQ_d = nc.dram_tensor("bQ", [NH * DH, 1], F32, kind="ExternalInput")
    bK_d = nc.dram_tensor("bK", [NH * DH, 1], F32, kind="ExternalInput")
    bV_d = nc.dram_tensor("bV", [1, VW], BF16, kind="ExternalInput")
    tri_d = nc.dram_tensor("tri", [P, P], BF16, kind="ExternalInput")
    ones_d = nc.dram_tensor("ones", [1, P], BF16, kind="ExternalInput")
    ones32_d = nc.dram_tensor("ones32", [1, P], F32, kind="ExternalInput")
    out_d = nc.dram_tensor("out", [S, EMB], F32, kind="ExternalOutput")

    with tile.TileContext(nc) as tc, nc.allow_low_precision(
        reason="bf16 attention; fp32 PSUM accumulation; rel-err gate is 2e-2"
    ):
        with (
            tc.tile_pool(name="c", bufs=1) as cp,
            tc.tile_pool(name="xp", bufs=1) as xp,
            tc.tile_pool(name="ex", bufs=PIPE + 1) as ex,
            tc.tile_pool(name="rc", bufs=2) as rc,
        ):
            # ---- persistent SBUF ----
            xT = [xp.tile([P, S], BF16, name=f"xT{i}", tag=f"xT{i}")
                  for i in range(NKT)]
            WQ = [cp.tile([P, NH * DH], BF16, name=f"WQ{i}", tag=f"WQ{i}")
                  for i in range(NKT)]
            WK = [cp.tile([P, NH * DH], BF16, name=f"WK{i}", tag=f"WK{i}")
                  for i in range(NKT)]
            WV = [cp.tile([P, VW], BF16, name=f"WV{i}", tag=f"WV{i}")
                  for i in range(NKT)]
            WO = [cp.tile([P, EMB], BF16, name=f"WO{i}", tag=f"WO{i}")
                  for i in range(NG)]
            QT = [cp.tile([P, S], BF16, name=f"QT{i}", tag=f"QT{i}")
                  for i in range(NG)]
            KT = [cp.tile([P, S], BF16, name=f"KT{i}", tag=f"KT{i}")
                  for i in range(NG)]
            zTn = [cp.tile([P, S], BF16, name=f"zTn{i}", tag=f"zTn{i}")
                   for i in range(NG)]
            Vp = cp.tile([P, NTT * VW], BF16, name="Vp", tag="Vp")
            bQ = cp.tile([P, NG], F32, name="bQ", tag="bQ")
            bK = cp.tile([P, NG], F32, name="bK", tag="bK")
            bV = cp.tile([1, VW], BF16, name="bV", tag="bV")
            tri = cp.tile([P, P], BF16, name="tri", tag="tri")
            ones = cp.tile([1, P], BF16, name="ones", tag="ones")
            ones32 = cp.tile([1, P], F32, name="ones32", tag="ones32")

            for i in range(NKT):
                nc.sync.dma_start(xT[i][:], xT_d[i * P : (i + 1) * P, :])
                nc.sync.dma_start(WQ[i][:], WQ_d[i * P : (i + 1) * P, :])
                nc.sync.dma_start(WK[i][:], WK_d[i * P : (i + 1) * P, :])
                nc.sync.dma_start(WV[i][:], WV_d[i * P : (i + 1) * P, :])
            for g in range(NG):
                nc.sync.dma_start(WO[g][:], WO_d[g * P : (g + 1) * P, :])
                nc.sync.dma_start(bQ[:, g : g + 1], bQ_d[g * P : (g + 1) * P, :])
                nc.sync.dma_start(bK[:, g : g + 1], bK_d[g * P : (g + 1) * P, :])
            nc.sync.dma_start(bV[:], bV_d[:])
            nc.sync.dma_start(tri[:], tri_d[:])
            nc.sync.dma_start(ones[:], ones_d[:])
            nc.sync.dma_start(ones32[:], ones32_d[:])

            # ---- projection + attention (PSUM pools scoped) ----
            psum_pools = (
                tc.tile_pool(name="pj", bufs=2, space="PSUM"),
                tc.tile_pool(name="sc", bufs=2, space="PSUM"),
                tc.tile_pool(name="za", bufs=1, space="PSUM"),
            )
            pj = psum_pools[0].__enter__()
            sc = psum_pools[1].__enter__()
            za = psum_pools[2].__enter__()

            # ---- emission helpers ----
            def emit_v_proj():
                for tt in range(NTT):
                    ps = pj.tile([P, TC], F32, name="pv", tag="pj")
                    for k in range(NKT):
                        nc.tensor.matmul(
                            ps[:, 0:VW],
                            xT[k][:, bass.ts(tt, P)],
                            WV[k][:],
                            start=(k == 0),
                            stop=False,
                        )
                    nc.tensor.matmul(ps[:, 0:VW], ones[:], bV[:],
                                     start=False, stop=True)
                    nc.vector.tensor_copy(
                        Vp[:, tt * VW : (tt + 1) * VW], ps[:, 0:VW]
                    )

            def proj_ops(g):
                # emission closures: one matmul (or evac) each, so the
                # scheduler can drip projection work between attention tiles
                ops = []
                for dst, W, b in ((QT, WQ, bQ), (KT, WK, bK)):
                    for t in range(NTC):
                        cell = {}

                        def mm(k, cell=cell, W=W, g=g, t=t):
                            if k == 0:
                                cell["ps"] = pj.tile(
                                    [P, TC], F32, name="pj", tag="pj"
                                )
                            nc.tensor.matmul(
                                cell["ps"][:],
                                W[k][:, g * P : (g + 1) * P],
                                xT[k][:, bass.ts(t, TC)],
                                start=(k == 0),
                                stop=(k == NKT - 1),
                            )

                        for k in range(NKT):
                            ops.append(lambda k=k, mm=mm: mm(k))

                        def evac(cell=cell, dst=dst, g=g, t=t, b=b):
                            nc.vector.tensor_scalar_add(
                                dst[g][:, bass.ts(t, TC)],
                                cell["ps"][:],
                                b[:, g : g + 1],
                            )

                        ops.append(evac)
                return ops

            def emit_qk_proj(g):
                for op in proj_ops(g):
                    op()

            def emit_qk(g, h2, kt):
                hs = slice(h2 * DH, (h2 + 1) * DH)
                q0 = kt * P
                nv = S - q0
                et = ex.tile([P, S], BF16, name="exp", tag="exp")
                for ci in range((nv + TC - 1) // TC):
                    c0 = q0 + ci * TC
                    cl = min(TC, S - c0)
                    sp = sc.tile([P, TC], F32, name="sc", tag="sc")
                    nc.tensor.matmul(
                        sp[:, 0:cl],
                        KT[g][hs, bass.ts(kt, P)],
                        QT[g][hs, c0 : c0 + cl],
                        start=True,
                        stop=True,
                    )
                    nc.scalar.activation(
                        et[:, ci * TC : ci * TC + cl],
                        sp[:, 0:cl],
                        mybir.ActivationFunctionType.Exp,
                        scale=0.125,
                    )
                # causal mask on the diagonal 128 columns
                nc.vector.tensor_mul(et[:, 0:P], et[:, 0:P], tri[:])
                return et

            def emit_pv(u, zt, kt, et):
                q0 = kt * P
                vsl = Vp[:, kt * VW + u * (DH + 1) :
                         kt * VW + (u + 1) * (DH + 1)]
                for qc in range(NTC):
                    lo = max(q0, qc * TC)
                    hi = (qc + 1) * TC
                    if lo >= hi:
                        continue
                    nc.tensor.matmul(
                        zt[qc][0 : DH + 1, lo - qc * TC : hi - qc * TC],
                        vsl,
                        et[:, lo - q0 : hi - q0],
                        start=(kt == 0),
                        stop=(kt == (hi - 1) // P),
                    )

            def emit_epilogue(g, h2, zt):
                hs = slice(h2 * DH, (h2 + 1) * DH)
                for qc in range(NTC):
                    rcp = rc.tile([1, TC], F32, name="rcp", tag="rcp")
                    nc.vector.reciprocal(rcp[:], zt[qc][DH : DH + 1, :])
                    rbt = sc.tile([P, TC], F32, name="rb", tag="sc")
                    nc.tensor.matmul(
                        rbt[0:DH, :],
                        ones32[:, 0:DH],
                        rcp[:],
                        start=True,
                        stop=True,
                    )
                    rbs = rc.tile([DH, TC], BF16, name="rbs", tag="rbs")
                    nc.vector.tensor_copy(rbs[:], rbt[0:DH, :])
                    nc.vector.tensor_mul(
                        zTn[g][hs, bass.ts(qc, TC)],
                        zt[qc][0:DH, :],
                        rbs[:],
                    )

            # ---- main schedule ----
            emit_v_proj()
            emit_qk_proj(0)
            prev = None  # (g, h2, zt) awaiting epilogue
            feeder = []  # next pair's projection ops, dripped ~2 per key tile
            for u in range(NH):
                g, h2 = u // 2, u % 2
                if h2 == 0 and g + 1 < NG:
                    feeder = proj_ops(g + 1)
                zt = [za.tile([P, TC], F32, name=f"z{qc}", tag=f"z{qc}")
                      for qc in range(NTC)]
                ets = {}
                for kt in range(NTT):
                    ets[kt] = emit_qk(g, h2, kt)
                    if kt == PIPE - 1 and prev is not None:
                        emit_epilogue(*prev)
                        prev = None
                    if kt >= PIPE:
                        emit_pv(u, zt, kt - PIPE, ets.pop(kt - PIPE))
                    for _ in range(2):
                        if feeder:
                            feeder.pop(0)()
                for kt in range(NTT - PIPE, NTT):
                    emit_pv(u, zt, kt, ets.pop(kt))
                prev = (g, h2, zt)
            while feeder:
                feeder.pop(0)()
            emit_epilogue(*prev)
            for _pp in reversed(psum_pools):
                _pp.__exit__(None, None, None)

            # ---- output projection (partial over this core's 6 heads) ----
            with (
                tc.tile_pool(name="op", bufs=2, space="PSUM") as op,
                tc.tile_pool(name="ob", bufs=3) as ob,
            ):
                for qt in range(NTT):
                    ot = op.tile([P, EMB], F32, name="op", tag="op")
                    for g in range(NG):
                        for c0, cl in ((0, 512), (512, 256)):
                            nc.tensor.matmul(
                                ot[:, c0 : c0 + cl],
                                zTn[g][:, bass.ts(qt, P)],
                                WO[g][:, c0 : c0 + cl],
                                start=(g == 0),
                                stop=(g == NG - 1),
                            )
                    obt = ob.tile([P, EMB], F32, name="ob", tag="ob")
                    nc.vector.tensor_copy(obt[:], ot[:])
                    nc.sync.dma_start(out_d[bass.ts(qt, P), :], obt[:])

    nc.compile()
    return nc


def _in_maps(x, W_Q, W_K, W_V, W_O, b_Q, b_K, b_V):
    maps = []
    for c in range(N_CORES):
        b, grp = c // 2, c % 2
        H = slice(grp * NH, (grp + 1) * NH)
        maps.append(
            {
                "xT": np.ascontiguousarray(x[b].T).astype(BF),
                "WQ": np.ascontiguousarray(
                    W_Q[H].transpose(1, 0, 2).reshape(EMB, NH * DH)
                ).astype(BF),
                "WK": np.ascontiguousarray(
                    W_K[H].transpose(1, 0, 2).reshape(EMB, NH * DH)
                ).astype(BF),
                "WV": _widen_wv(W_V[H]),
                "WO": np.ascontiguousarray(W_O[H].reshape(NH * DH, EMB)).astype(BF),
                "bQ": np.ascontiguousarray(b_Q[H].reshape(NH * DH, 1)),
                "bK": np.ascontiguousarray(b_K[H].reshape(NH * DH, 1)),
                "bV": _widen_bv(b_V[H]),
                "tri": _TRI,
                "ones": _ONES,
                "ones32": _ONES.astype(np.float32),
            }
        )
    return maps


def run(x, W_Q, W_K, W_V, W_O, b_Q, b_K, b_V, b_O, trace=False):
    if "nc" not in _graph_cache:
        _graph_cache["nc"] = build_graph()
    nc = _graph_cache["nc"]
    res = run_bass_kernel_spmd(
        nc,
        _in_maps(x, W_Q, W_K, W_V, W_O, b_Q, b_K, b_V),
        core_ids=list(range(N_CORES)),
        trace=trace,
    )
    parts = [res.results[c]["out"] for c in range(N_CORES)]
    out = np.stack(
        [parts[2 * b] + parts[2 * b + 1] + b_O[None, :] for b in range(4)]
    ).astype(np.float32)
    return out, res


def kernel(**inputs):
    out, _ = run(**inputs)
    return out


# revision 24
# speedup vs baseline: 1.1699x; 1.1699x over previous
"""Causal MHA on 8 TRN2 NeuronCores.

Sharding: 8 cores = 4 batches x 2 head-groups (6 heads each).  Every core
runs an identical graph: QKV projections for its batch + head-group, causal
attention (max-free softmax, scores kept key-major so exp output feeds the
PV matmul directly with no transpose), and a partial output projection over
its 6 heads.  Host sums the two partials per batch (+ b_O).  No collectives.

Schedule: V projection up front, then Q/K projections for head-pair g+1 are
interleaved into the attention of pair g, keeping TensorE dense while
ScalarE chews on exp (which keeps the HAM clock-gate at 2.4 GHz).  A 2-deep
software pipeline (PV trails QK) hides exp latency inside each unit, and
each unit's softmax epilogue is emitted during the next unit's QK prologue.

All matmul operands are bf16 (1 cycle/row at any N, FWL weight loads);
PSUM accumulation stays fp32.  exp is max-free (scores are bounded), and
an extra all-ones V' column makes the PV matmul emit softmax denominators.
"""

import sys

for _p in ("/opt/trn_rl_repo",):
    if _p not in sys.path:
        sys.path.insert(0, _p)

import ml_dtypes
import numpy as np

import concourse.bass as bass
import concourse.mybir as mybir
import concourse.tile as tile
from concourse import bacc
from concourse.bass_utils import run_bass_kernel_spmd

F32 = mybir.dt.float32
BF16 = mybir.dt.bfloat16
BF = ml_dtypes.bfloat16

P = 128
EMB = 768
NKT = EMB // P          # 6 emb tiles
S = 2048
NTT = S // P            # 16 token tiles
DH = 64
NH = 6                  # heads per core
NG = NH // 2            # dh-groups of 128 (2 heads stacked)
TC = 512
NTC = S // TC           # 4 token chunks
VW = NH * (DH + 1)      # 390: V' row width per token tile
N_CORES = 8
PIPE = 2                # PV trails QK by this many key tiles

_graph_cache = {}

_TRI = np.triu(np.ones((P, P))).astype(BF)  # tri[k, q] = 1 iff k <= q
_ONES = np.ones((1, P), dtype=BF)


def _widen_wv(wv):  # [NH, EMB, DH] -> [EMB, NH*(DH+1)], zero 65th cols
    out = np.zeros((EMB, VW), dtype=BF)
    out.reshape(EMB, NH, DH + 1)[:, :, :DH] = wv.transpose(1, 0, 2).astype(BF)
    return out


def _widen_bv(bv):  # [NH, DH] -> [1, NH*(DH+1)], 1.0 at 65th cols
    out = np.zeros((1, VW), dtype=BF)
    out.reshape(NH, DH + 1)[:, :DH] = bv.astype(BF)
    out.reshape(NH, DH + 1)[:, DH] = 1.0
    return out


def build_graph():
    nc = bacc.Bacc(None, target_bir_lowering=False)

    xT_d = nc.dram_tensor("xT", [EMB, S], BF16, kind="ExternalInput")
    WQ_d = nc.dram_tensor("WQ", [EMB, NH * DH], BF16, kind="ExternalInput")
    WK_d = nc.dram_tensor("WK", [EMB, NH * DH], BF16, kind="ExternalInput")
    WV_d = nc.dram_tensor("WV", [EMB, VW], BF16, kind="ExternalInput")
    WO_d = nc.dram_tensor("WO", [NH * DH, EMB], BF16, kind="ExternalInput")
    bQ_d = nc.dram_tensor("bQ", [NH * DH, 1], F32, kind="ExternalInput")
    bK_d = nc.dram_tensor("bK", [NH * DH, 1], F32, kind="ExternalInput")
    bV_d = nc.dram_tensor("bV", [1, VW], BF16, kind="ExternalInput")
    tri_d = nc.dram_tensor("tri", [P, P], BF16, kind="ExternalInput")
    ones_d = nc.dram_tensor("ones", [1, P], BF16, kind="ExternalInput")
    ones32_d = nc.dram_tensor("ones32", [1, P], F32, kind="ExternalInput")
    out_d = nc.dram_tensor("out", [S, EMB], F32, kind="ExternalOutput")

    with tile.TileContext(nc) as tc, nc.allow_low_precision(
        reason="bf16 attention; fp32 PSUM accumulation; rel-err gate is 2e-2"
    ):
        with (
            tc.tile_pool(name="c", bufs=1) as cp,
            tc.tile_pool(name="xp", bufs=1) as xp,
            tc.tile_pool(name="ex", bufs=PIPE + 1) as ex,
            tc.tile_pool(name="rc", bufs=2) as rc,
        ):
            # ---- persistent SBUF ----
            xT = [xp.tile([P, S], BF16, name=f"xT{i}", tag=f"xT{i}")
                  for i in range(NKT)]
            WQ = [cp.tile([P, NH * DH], BF16, name=f"WQ{i}", tag=f"WQ{i}")
                  for i in range(NKT)]
            WK = [cp.tile([P, NH * DH], BF16, name=f"WK{i}", tag=f"WK{i}")
                  for i in range(NKT)]
            WV = [cp.tile([P, VW], BF16, name=f"WV{i}", tag=f"WV{i}")
                  for i in range(NKT)]
            WO = [cp.tile([P, EMB], BF16, name=f"WO{i}", tag=f"WO{i}")
                  for i in range(NG)]
            QT = [cp.tile([P, S], BF16, name=f"QT{i}", tag=f"QT{i}")
                  for i in range(NG)]
            KT = [cp.tile([P, S], BF16, name=f"KT{i}", tag=f"KT{i}")
                  for i in range(NG)]
            zTn = [cp.tile([P, S], BF16, name=f"zTn{i}", tag=f"zTn{i}")
                   for i in range(NG)]
            Vp = cp.tile([P, NTT * VW], BF16, name="Vp", tag="Vp")
            bQ = cp.tile([P, NG], F32, name="bQ", tag="bQ")
            bK = cp.tile([P, NG], F32, name="bK", tag="bK")
            bV = cp.tile([1, VW], BF16, name="bV", tag="bV")
            tri = cp.tile([P, P], BF16, name="tri", tag="tri")
            ones = cp.tile([1, P], BF16, name="ones", tag="ones")
            ones32 = cp.tile([1, P], F32, name="ones32", tag="ones32")

            for i in range(NKT):
                nc.sync.dma_start(xT[i][:], xT_d[i * P : (i + 1) * P, :])
                nc.sync.dma_start(WQ[i][:], WQ_d[i * P : (i + 1) * P, :])
                nc.sync.dma_start(WK[i][:], WK_d[i * P : (i + 1) * P, :])
                nc.sync.dma_start(WV[i][:], WV_d[i * P : (i + 1) * P, :])
            for g in range(NG):
                nc.sync.dma_start(WO[g][:], WO_d[g * P : (g + 1) * P, :])
                nc.sync.dma_start(bQ[:, g : g + 1], bQ_d[g * P : (g + 1) * P, :])
                nc.sync.dma_start(bK[:, g : g + 1], bK_d[g * P : (g + 1) * P, :])
            nc.sync.dma_start(bV[:], bV_d[:])
            nc.sync.dma_start(tri[:], tri_d[:])
            nc.sync.dma_start(ones[:], ones_d[:])
            nc.sync.dma_start(ones32[:], ones32_d[:])

            # ---- projection + attention (PSUM pools scoped) ----
            pj_cm = tc.tile_pool(name="pj", bufs=2, space="PSUM")
            pj = pj_cm.__enter__()
            pools = {}  # sc/za opened after projections (PSUM is LIFO-scoped)

            # ---- emission helpers ----
            def emit_v_proj():
                for tt in range(NTT):
                    ps = pj.tile([P, TC], F32, name="pv", tag="pj")
                    for k in range(NKT):
                        nc.tensor.matmul(
                            ps[:, 0:VW],
                            xT[k][:, bass.ts(tt, P)],
                            WV[k][:],
                            start=(k == 0),
                            stop=False,
                        )
                    nc.tensor.matmul(ps[:, 0:VW], ones[:], bV[:],
                                     start=False, stop=True)
                    nc.vector.tensor_copy(
                        Vp[:, tt * VW : (tt + 1) * VW], ps[:, 0:VW]
                    )

            def proj_ops(g):
                # emission closures: one matmul (or evac) each, so the
                # scheduler can drip projection work between attention tiles
                ops = []
                for dst, W, b in ((QT, WQ, bQ), (KT, WK, bK)):
                    for t in range(NTC):
                        cell = {}

                        def mm(k, cell=cell, W=W, g=g, t=t):
                            if k == 0:
                                cell["ps"] = pj.tile(
                                    [P, TC], F32, name="pj", tag="pj"
                                )
                            nc.tensor.matmul(
                                cell["ps"][:],
                                W[k][:, g * P : (g + 1) * P],
                                xT[k][:, bass.ts(t, TC)],
                                start=(k == 0),
                                stop=(k == NKT - 1),
                            )

                        for k in range(NKT):
                            ops.append(lambda k=k, mm=mm: mm(k))

                        def evac(cell=cell, dst=dst, g=g, t=t, b=b):
                            nc.vector.tensor_scalar_add(
                                dst[g][:, bass.ts(t, TC)],
                                cell["ps"][:],
                                b[:, g : g + 1],
                            )

                        ops.append(evac)
                return ops

            def emit_qk_proj(g):
                for op in proj_ops(g):
                    op()

            def emit_qk(g, h2, kt):
                hs = slice(h2 * DH, (h2 + 1) * DH)
                q0 = kt * P
                nv = S - q0
                et = ex.tile([P, S], BF16, name="exp", tag="exp")
                EC = 2 * TC  # exp chunk: 2 PSUM banks, one ACT instruction
                for ci in range((nv + EC - 1) // EC):
                    c0 = q0 + ci * EC
                    cl = min(EC, S - c0)
                    sp = pools['sc'].tile([P, EC], F32, name="sc", tag="sc")
                    for mi in range(0, cl, TC):
                        ml = min(TC, cl - mi)
                        nc.tensor.matmul(
                            sp[:, mi : mi + ml],
                            KT[g][hs, bass.ts(kt, P)],
                            QT[g][hs, c0 + mi : c0 + mi + ml],
                            start=True,
                            stop=True,
                        )
                    nc.scalar.activation(
                        et[:, ci * EC : ci * EC + cl],
                        sp[:, 0:cl],
                        mybir.ActivationFunctionType.Exp,
                        scale=0.125,
                    )
                # causal mask on the diagonal 128 columns
                nc.vector.tensor_mul(et[:, 0:P], et[:, 0:P], tri[:])
                return et

            def emit_pv(u, zt, kt, et):
                q0 = kt * P
                vsl = Vp[:, kt * VW + u * (DH + 1) :
                         kt * VW + (u + 1) * (DH + 1)]
                for qc in range(NTC):
                    lo = max(q0, qc * TC)
                    hi = (qc + 1) * TC
                    if lo >= hi:
                        continue
                    nc.tensor.matmul(
                        zt[qc][0 : DH + 1, lo - qc * TC : hi - qc * TC],
                        vsl,
                        et[:, lo - q0 : hi - q0],
                        start=(kt == 0),
                        stop=(kt == (hi - 1) // P),
                    )

            def emit_epilogue(g, h2, zt):
                hs = slice(h2 * DH, (h2 + 1) * DH)
                for qc in range(NTC):
                    srow = rc.tile([1, TC], F32, name="srow", tag="srow")
                    nc.vector.tensor_copy(srow[:], zt[qc][DH : DH + 1, :])
                    rcp32 = rc.tile([1, TC], F32, name="rcp32", tag="rcp32")
                    nc.vector.reciprocal_approx_fast(rcp32[:], srow[:])
                    rcp = rc.tile([1, TC], BF16, name="rcp", tag="rcp")
                    nc.vector.tensor_copy(rcp[:], rcp32[:])
                    rbt = pools['sc'].tile([P, 2 * TC], F32, name="rb", tag="sc")
                    nc.tensor.matmul(
                        rbt[0:DH, 0:TC],
                        ones[:, 0:DH],
                        rcp[:],
                        start=True,
                        stop=True,
                    )
                    rbs = rc.tile([DH, TC], BF16, name="rbs", tag="rbs")
                    nc.vector.tensor_copy(rbs[:], rbt[0:DH, 0:TC])
                    nc.vector.tensor_mul(
                        zTn[g][hs, bass.ts(qc, TC)],
                        zt[qc][0:DH, :],
                        rbs[:],
                    )

            # ---- main schedule ----
            emit_v_proj()
            for g in range(NG):
                emit_qk_proj(g)
            pj_cm.__exit__(None, None, None)
            psum_pools = (
                tc.tile_pool(name="sc", bufs=2, space="PSUM"),
                tc.tile_pool(name="za", bufs=1, space="PSUM"),
            )
            pools['sc'] = psum_pools[0].__enter__()
            pools['za'] = psum_pools[1].__enter__()
            prev = None  # (g, h2, zt) awaiting epilogue
            for u in range(NH):
                g, h2 = u // 2, u % 2
                zt = [pools['za'].tile([P, TC], F32, name=f"z{qc}", tag=f"z{qc}")
                      for qc in range(NTC)]
                ets = {}
                for kt in range(NTT):
                    ets[kt] = emit_qk(g, h2, kt)
                    if kt == PIPE - 1 and prev is not None:
                        emit_epilogue(*prev)
                        prev = None
                    if kt >= PIPE:
                        emit_pv(u, zt, kt - PIPE, ets.pop(kt - PIPE))
                for kt in range(NTT - PIPE, NTT):
                    emit_pv(u, zt, kt, ets.pop(kt))
                prev = (g, h2, zt)
            emit_epilogue(*prev)
            for _pp in reversed(psum_pools):
                _pp.__exit__(None, None, None)

            # ---- output projection (partial over this core's 6 heads) ----
            with (
                tc.tile_pool(name="op", bufs=2, space="PSUM") as op,
                tc.tile_pool(name="ob", bufs=3) as ob,
            ):
                for qt in range(NTT):
                    ot = op.tile([P, EMB], F32, name="op", tag="op")
                    for g in range(NG):
                        for c0, cl in ((0, 512), (512, 256)):
                            nc.tensor.matmul(
                                ot[:, c0 : c0 + cl],
                                zTn[g][:, bass.ts(qt, P)],
                                WO[g][:, c0 : c0 + cl],
                                start=(g == 0),
                                stop=(g == NG - 1),
                            )
                    obt = ob.tile([P, EMB], F32, name="ob", tag="ob")
                    nc.vector.tensor_copy(obt[:], ot[:])
                    nc.sync.dma_start(out_d[bass.ts(qt, P), :], obt[:])

    nc.compile()
    return nc


def _in_maps(x, W_Q, W_K, W_V, W_O, b_Q, b_K, b_V):
    maps = []
    for c in range(N_CORES):
        b, grp = c // 2, c % 2
        H = slice(grp * NH, (grp + 1) * NH)
        maps.append(
            {
                "xT": np.ascontiguousarray(x[b].T).astype(BF),
                "WQ": np.ascontiguousarray(
                    W_Q[H].transpose(1, 0, 2).reshape(EMB, NH * DH)
                ).astype(BF),
                "WK": np.ascontiguousarray(
                    W_K[H].transpose(1, 0, 2).reshape(EMB, NH * DH)
                ).astype(BF),
                "WV": _widen_wv(W_V[H]),
                "WO": np.ascontiguousarray(W_O[H].reshape(NH * DH, EMB)).astype(BF),
                "bQ": np.ascontiguousarray(b_Q[H].reshape(NH * DH, 1)),
                "bK": np.ascontiguousarray(b_K[H].reshape(NH * DH, 1)),
                "bV": _widen_bv(b_V[H]),
                "tri": _TRI,
                "ones": _ONES,
                "ones32": _ONES.astype(np.float32),
            }
        )
    return maps


def run(x, W_Q, W_K, W_V, W_O, b_Q, b_K, b_V, b_O, trace=False):
    if "nc" not in _graph_cache:
        _graph_cache["nc"] = build_graph()
    nc = _graph_cache["nc"]
    res = run_bass_kernel_spmd(
        nc,
        _in_maps(x, W_Q, W_K, W_V, W_O, b_Q, b_K, b_V),
        core_ids=list(range(N_CORES)),
        trace=trace,
    )
    parts = [res.results[c]["out"] for c in range(N_CORES)]
    out = np.stack(
        [parts[2 * b] + parts[2 * b + 1] + b_O[None, :] for b in range(4)]
    ).astype(np.float32)
    return out, res


def kernel(**inputs):
    out, _ = run(**inputs)
    return out
